# revision 6
# baseline (speedup 1.0000x reference)
"""Multi-head self-attention TRN2 Bass kernel.

Problem: B=4, S=2048, EMB=1024, H=16 heads, dqk=dv=64.
Sharding: 8 cores; core c handles batch b=c//2 and head group g=c%2
(8 heads each). Each core computes its partial output projection
(rows of wo for its heads); host sums the two partials per batch and
adds bo.

Per-core inputs (bf16 except biases): xq_T/xkv_T [1024, 2048] (x.T),
wq/wk/wv [1024, 512], bq/bk/bv [512] f32, wo_g [512, 1024].
Output: partial out [2048, 1024] f32.

Dataflow:
  - Q/K projections j-major: QHT/KHT [512 j, 2048 s] bf16.
  - V projection t-major: VH [2048 t, 512 j] -> tiles [128, 8*66] with
    per-head blocks [64 V | 1 ones | 1 pad]; the ones column makes the
    AV matmul emit softmax denominators as row 64 of its output.
  - scores (t-major, K=64 row-tiled): S^T [128 t, 1024 s] PSUM, exp on
    ACT (scale=1/8 folded; max-free softmax, |scores| <~ 3) -> P^T bf16.
  - AV: accumulate Z~T [66, 512] over 16 t-tiles in PSUM.
  - normalize: reciprocal of D row, DMA round-trip broadcast across
    partitions, multiply + bv bias -> ZnormT [512 j, 2048 s] bf16.
  - out projection bf16 -> partial OUT [2048, 1024] f32 -> DRAM.
"""

import ml_dtypes
import numpy as np

import concourse.bass as bass
import concourse.tile as tile
from concourse import bacc, mybir
from concourse.bass_utils import run_bass_kernel_spmd

B, S, EMB, H, DH = 4, 2048, 1024, 16, 64
N_CORES = 8
HPC = H // 2          # heads per core
JC = HPC * DH         # 512: per-core projected width
VB = DH + 2           # 66: per-head V block (64 V cols + ones + pad)

F32 = mybir.dt.float32
BF16 = mybir.dt.bfloat16


def build_kernel():
    nc = bacc.Bacc(
        "TRN2", target_bir_lowering=False, debug=False, num_devices=N_CORES
    )

    xq = nc.dram_tensor("xq", [EMB, S], BF16, kind="ExternalInput").ap()
    xkv = nc.dram_tensor("xkv", [EMB, S], BF16, kind="ExternalInput").ap()
    wq_d = nc.dram_tensor("wq", [EMB, JC], BF16, kind="ExternalInput").ap()
    wk_d = nc.dram_tensor("wk", [EMB, JC], BF16, kind="ExternalInput").ap()
    wv_d = nc.dram_tensor("wv", [EMB, JC], BF16, kind="ExternalInput").ap()
    bq_d = nc.dram_tensor("bq", [JC], F32, kind="ExternalInput").ap()
    bk_d = nc.dram_tensor("bk", [JC], F32, kind="ExternalInput").ap()
    bv_d = nc.dram_tensor("bv", [JC], F32, kind="ExternalInput").ap()
    wo_d = nc.dram_tensor("wo", [JC, EMB], BF16, kind="ExternalInput").ap()
    out_d = nc.dram_tensor("out", [S, EMB], F32, kind="ExternalOutput").ap()
    dr_d = nc.dram_tensor("dr_scratch", [S], F32).ap()  # Drecip bounce

    with tile.TileContext(nc) as tc:
        with (
            tc.tile_pool(name="persist", bufs=1) as pp,
            tc.tile_pool(name="dreb_p", bufs=1) as dre_pool,
            tc.tile_pool(name="znsc", bufs=1) as zns_pool,
            tc.tile_pool(name="drec_p", bufs=4) as drec_pool,
        ):
            # persistent SBUF tensors
            qht = [pp.tile([128, S], BF16, name=f"qht{i}") for i in range(4)]
            kht = [pp.tile([128, S], BF16, name=f"kht{i}") for i in range(4)]
            vh = [pp.tile([128, HPC * VB], BF16, name=f"vh{t}")
                  for t in range(16)]
            znorm = [pp.tile([128, S], BF16, name=f"zn{i}") for i in range(4)]
            wos = [pp.tile([128, EMB], BF16, name=f"wo{j}") for j in range(4)]
            bias_q = pp.tile([128, 4], F32, name="bias_q")
            bias_k = pp.tile([128, 4], F32, name="bias_k")
            bias_v = pp.tile([64, HPC], F32, name="bias_v")  # [d, head]

            nc.sync.dma_start(bias_q[:], bq_d.rearrange("(c p) -> p c", p=128))
            nc.sync.dma_start(bias_k[:], bk_d.rearrange("(c p) -> p c", p=128))
            nc.sync.dma_start(bias_v[:], bv_d.rearrange("(h d) -> d h", d=DH))
            for j in range(4):
                nc.sync.dma_start(wos[j][:], wo_d[j * 128:(j + 1) * 128, :])
            # ones columns in vh blocks (col 64 of each 66-block); pad col 0
            for t in range(16):
                blocks = vh[t][:].rearrange("p (h c) -> p h c", c=VB)
                nc.vector.memset(blocks[:, :, DH:DH + 1], 1.0)
                nc.vector.memset(blocks[:, :, DH + 1:], 0.0)

            # ---------------- projections ----------------
            with (
                tc.tile_pool(name="xq_p", bufs=12) as xq_pool,
                tc.tile_pool(name="xkv_p", bufs=12) as xkv_pool,
                tc.tile_pool(name="wqkv", bufs=1) as w_pool,
                tc.tile_pool(name="proj_ps", bufs=4, space="PSUM") as proj_ps,
            ):
                wqs = [w_pool.tile([128, JC], BF16, name=f"wqs{e}")
                       for e in range(8)]
                wks = [w_pool.tile([128, JC], BF16, name=f"wks{e}")
                       for e in range(8)]
                wvs = [w_pool.tile([128, JC], BF16, name=f"wvs{e}")
                       for e in range(8)]
                for e in range(8):
                    nc.sync.dma_start(wqs[e][:], wq_d[e * 128:(e + 1) * 128, :])
                    nc.sync.dma_start(wks[e][:], wk_d[e * 128:(e + 1) * 128, :])
                    nc.sync.dma_start(wvs[e][:], wv_d[e * 128:(e + 1) * 128, :])

                for sc in range(4):
                    ssl = slice(sc * 512, (sc + 1) * 512)
                    xqs = []
                    xks = []
                    for e in range(8):
                        xt = xq_pool.tile([128, 512], BF16,
                                          tag="xq", name="xq")
                        nc.sync.dma_start(xt[:], xq[e * 128:(e + 1) * 128, ssl])
                        xqs.append(xt)
                        kt = xkv_pool.tile([128, 512], BF16,
                                           tag="xkv", name="xkv")
                        nc.sync.dma_start(kt[:],
                                          xkv[e * 128:(e + 1) * 128, ssl])
                        xks.append(kt)
                    # Q and K projections (j-major)
                    for dst, ws, xs, bias in (
                        (qht, wqs, xqs, bias_q),
                        (kht, wks, xks, bias_k),
                    ):
                        for jc in range(4):
                            ps = proj_ps.tile([128, 512], F32,
                                              tag="pps", name="pps")
                            jsl = slice(jc * 128, (jc + 1) * 128)
                            for e in range(8):
                                nc.tensor.matmul(
                                    ps[:],
                                    ws[e][:, jsl],
                                    xs[e][:],
                                    start=(e == 0),
                                    stop=(e == 7),
                                )
                            nc.vector.tensor_scalar_add(
                                dst[jc][:, ssl], ps[:], bias[:, jc:jc + 1]
                            )
                    # V projection (t-major)
                    for tl in range(4):
                        tch = sc * 4 + tl
                        ps = proj_ps.tile([128, 512], F32,
                                          tag="pps", name="pps")
                        tsl = slice(tl * 128, (tl + 1) * 128)
                        for e in range(8):
                            nc.tensor.matmul(
                                ps[:],
                                xks[e][:, tsl],
                                wvs[e][:],
                                start=(e == 0),
                                stop=(e == 7),
                            )
                        nc.vector.tensor_copy(
                            vh[tch][:].rearrange(
                                "p (h c) -> p h c", c=VB)[:, :, 0:DH],
                            ps[:].rearrange("p (h d) -> p h d", d=DH),
                        )

            # ---------------- attention ----------------
            with (
                tc.tile_pool(name="sps", bufs=2, space="PSUM") as sp_pool,
                tc.tile_pool(name="avps", bufs=1, space="PSUM") as av_pool,
                tc.tile_pool(name="pt", bufs=4) as pt_pool,
            ):
                for h in range(HPC):
                    pair = h // 2
                    off = (h % 2) * 64
                    avs = [
                        av_pool.tile([VB, 512], F32, tag=f"av{sc}",
                                     name=f"av{sc}")
                        for sc in range(4)
                    ]
                    for t in range(16):
                        lhs_s = kht[pair][off:off + 64, t * 128:(t + 1) * 128]
                        for half in range(2):
                            sp = sp_pool.tile([128, 1024], F32,
                                              tag="sp", name="sp")
                            for i in range(2):
                                sc = half * 2 + i
                                nc.tensor.matmul(
                                    sp[:, i * 512:(i + 1) * 512],
                                    lhs_s,
                                    qht[pair][off:off + 64,
                                              sc * 512:(sc + 1) * 512],
                                    start=True,
                                    stop=True,
                                )
                            ptt = pt_pool.tile([128, 1024], BF16,
                                               tag="ptt", name="ptt")
                            nc.scalar.activation(
                                ptt[:], sp[:],
                                mybir.ActivationFunctionType.Exp,
                                scale=0.125,
                            )
                            for i in range(2):
                                sc = half * 2 + i
                                nc.tensor.matmul(
                                    avs[sc][:],
                                    vh[t][:, h * VB:(h + 1) * VB],
                                    ptt[:, i * 512:(i + 1) * 512],
                                    start=(t == 0),
                                    stop=(t == 15),
                                    skip_group_check=True,
                                )
                    # normalize head: recip of D row, broadcast, mul, +bv
                    for sc in range(4):
                        drc = drec_pool.tile([66, 512], F32,
                                             tag="drc", name="drc")
                        nc.vector.reciprocal(
                            drc[64:65, :], avs[sc][DH:DH + 1, :]
                        )
                        nc.sync.dma_start(
                            dr_d[sc * 512:(sc + 1) * 512], drc[64:65, :]
                        )
                    dreb = dre_pool.tile([64, S], F32, tag="dreb", name="dreb")
                    nc.sync.dma_start(
                        dreb[:], dr_d.unsqueeze(0).broadcast_to([64, S])
                    )
                    zn_s = zns_pool.tile([64, S], BF16, tag="zn_s", name="zn_s")
                    for sc in range(4):
                        nc.vector.tensor_mul(
                            zn_s[:, sc * 512:(sc + 1) * 512],
                            avs[sc][0:DH, :],
                            dreb[:, sc * 512:(sc + 1) * 512],
                        )
                    nc.vector.tensor_scalar_add(
                        zn_s[:], zn_s[:], bias_v[:, h:h + 1]
                    )
                    nc.sync.dma_start(znorm[pair][off:off + 64, :], zn_s[:])

            # ---------------- output projection ----------------
            with (
                tc.tile_pool(name="ops", bufs=4, space="PSUM") as op_pool,
                tc.tile_pool(name="ostg", bufs=4) as ostg_pool,
            ):
                for scc in range(16):
                    psl = slice(scc * 128, (scc + 1) * 128)
                    for oc in range(2):
                        ps = op_pool.tile([128, 512], F32, tag="ops",
                                          name="ops")
                        osl = slice(oc * 512, (oc + 1) * 512)
                        for jt in range(4):
                            nc.tensor.matmul(
                                ps[:],
                                znorm[jt][:, psl],
                                wos[jt][:, osl],
                                start=(jt == 0),
                                stop=(jt == 3),
                            )
                        ostg = ostg_pool.tile([128, 512], F32, tag="ostg",
                                              name="ostg")
                        nc.vector.tensor_copy(ostg[:], ps[:])
                        nc.sync.dma_start(out_d[psl, osl], ostg[:])

    nc.compile()
    return nc


def _bf16(a):
    return np.asarray(a, np.float32).astype(ml_dtypes.bfloat16)


def _prep_inputs(q, k_and_v, wq, bq, wk, bk, wv, bv, wo):
    """Build per-core input maps."""
    in_maps = []
    for c in range(N_CORES):
        b, g = c // 2, c % 2
        hs = slice(g * HPC, (g + 1) * HPC)
        # [H, emb, d] -> [emb, H*d] for this head group
        wq_g = np.transpose(wq[hs], (1, 0, 2)).reshape(EMB, JC)
        wk_g = np.transpose(wk[hs], (1, 0, 2)).reshape(EMB, JC)
        wv_g = np.transpose(wv[hs], (1, 0, 2)).reshape(EMB, JC)
        in_maps.append({
            "xq": np.ascontiguousarray(_bf16(q[b]).T),
            "xkv": np.ascontiguousarray(_bf16(k_and_v[b]).T),
            "wq": np.ascontiguousarray(_bf16(wq_g)),
            "wk": np.ascontiguousarray(_bf16(wk_g)),
            "wv": np.ascontiguousarray(_bf16(wv_g)),
            "bq": np.ascontiguousarray(np.asarray(bq, np.float32)[hs]
                                       .reshape(JC)),
            "bk": np.ascontiguousarray(np.asarray(bk, np.float32)[hs]
                                       .reshape(JC)),
            "bv": np.ascontiguousarray(np.asarray(bv, np.float32)[hs]
                                       .reshape(JC)),
            "wo": np.ascontiguousarray(
                _bf16(wo)[g * JC:(g + 1) * JC, :]),
        })
    return in_maps


_NC_CACHE = {}


def kernel(q, k_and_v, wq, bq, wk, bk, wv, bv, wo, bo):
    if "nc" not in _NC_CACHE:
        _NC_CACHE["nc"] = build_kernel()
    nc = _NC_CACHE["nc"]
    in_maps = _prep_inputs(q, k_and_v, wq, bq, wk, bk, wv, bv, wo)
    res = run_bass_kernel_spmd(nc, in_maps, core_ids=list(range(N_CORES)))
    bo = np.asarray(bo, np.float32)
    out = np.empty((B, S, EMB), np.float32)
    for b in range(B):
        out[b] = res.results[2 * b]["out"] + res.results[2 * b + 1]["out"] + bo
    return out


# revision 7
# speedup vs baseline: 132.4776x; 132.4776x over previous
"""Multi-head self-attention TRN2 Bass kernel.

Problem: B=4, S=2048, EMB=1024, H=16 heads, dqk=dv=64.
Sharding: 8 cores; core c handles batch b=c//2 and head group g=c%2
(8 heads each). Each core computes its partial output projection
(rows of wo for its heads); host sums the two partials per batch and
adds bo.

Per-core inputs (bf16 except biases): xq_T/xkv_T [1024, 2048] (x.T),
wq/wk/wv [1024, 512], bq/bk/bv [512] f32, wo_g [512, 1024].
Output: partial out [2048, 1024] f32.

Dataflow:
  - Q/K projections j-major: QHT/KHT [512 j, 2048 s] bf16.
  - V projection t-major: VH [2048 t, 512 j] -> tiles [128, 8*66] with
    per-head blocks [64 V | 1 ones | 1 pad]; the ones column makes the
    AV matmul emit softmax denominators as row 64 of its output.
  - scores (t-major, K=64 row-tiled): S^T [128 t, 1024 s] PSUM, exp on
    ACT (scale=1/8 folded; max-free softmax, |scores| <~ 3) -> P^T bf16.
  - AV: accumulate Z~T [66, 512] over 16 t-tiles in PSUM.
  - normalize: reciprocal of D row, DMA round-trip broadcast across
    partitions, multiply + bv bias -> ZnormT [512 j, 2048 s] bf16.
  - out projection bf16 -> partial OUT [2048, 1024] f32 -> DRAM.
"""

import ml_dtypes
import numpy as np

import concourse.bass as bass
import concourse.tile as tile
from concourse import bacc, mybir
from concourse.bass_utils import run_bass_kernel_spmd

B, S, EMB, H, DH = 4, 2048, 1024, 16, 64
N_CORES = 8
HPC = H // 2          # heads per core
JC = HPC * DH         # 512: per-core projected width
VB = DH + 2           # 66: per-head V block (64 V cols + ones + pad)

F32 = mybir.dt.float32
BF16 = mybir.dt.bfloat16


def build_kernel(reps=1):
    nc = bacc.Bacc(
        "TRN2", target_bir_lowering=False, debug=False, num_devices=N_CORES
    )

    xq = nc.dram_tensor("xq", [EMB, S], BF16, kind="ExternalInput").ap()
    xkv = nc.dram_tensor("xkv", [EMB, S], BF16, kind="ExternalInput").ap()
    wq_d = nc.dram_tensor("wq", [EMB, JC], BF16, kind="ExternalInput").ap()
    wk_d = nc.dram_tensor("wk", [EMB, JC], BF16, kind="ExternalInput").ap()
    wv_d = nc.dram_tensor("wv", [EMB, JC], BF16, kind="ExternalInput").ap()
    bq_d = nc.dram_tensor("bq", [JC], F32, kind="ExternalInput").ap()
    bk_d = nc.dram_tensor("bk", [JC], F32, kind="ExternalInput").ap()
    bv_d = nc.dram_tensor("bv", [JC], F32, kind="ExternalInput").ap()
    wo_d = nc.dram_tensor("wo", [JC, EMB], BF16, kind="ExternalInput").ap()
    out_d = nc.dram_tensor("out", [S, EMB], F32, kind="ExternalOutput").ap()
    dr_d = nc.dram_tensor("dr_scratch", [S], F32).ap()  # Drecip bounce

    import contextlib

    with tile.TileContext(nc) as tc:
        with (
            tc.For_i(0, reps, 1) if reps > 1 else contextlib.nullcontext(),
            tc.tile_pool(name="persist", bufs=1) as pp,
            tc.tile_pool(name="dreb_p", bufs=1) as dre_pool,
            tc.tile_pool(name="znsc", bufs=1) as zns_pool,
            tc.tile_pool(name="drec_p", bufs=4) as drec_pool,
        ):
            # persistent SBUF tensors
            qht = [pp.tile([128, S], BF16, name=f"qht{i}") for i in range(4)]
            kht = [pp.tile([128, S], BF16, name=f"kht{i}") for i in range(4)]
            vh = [pp.tile([128, HPC * VB], BF16, name=f"vh{t}")
                  for t in range(16)]
            znorm = [pp.tile([128, S], BF16, name=f"zn{i}") for i in range(4)]
            wos = [pp.tile([128, EMB], BF16, name=f"wo{j}") for j in range(4)]
            bias_q = pp.tile([128, 4], F32, name="bias_q")
            bias_k = pp.tile([128, 4], F32, name="bias_k")
            bias_v = pp.tile([64, HPC], F32, name="bias_v")  # [d, head]

            nc.sync.dma_start(bias_q[:], bq_d.rearrange("(c p) -> p c", p=128))
            nc.sync.dma_start(bias_k[:], bk_d.rearrange("(c p) -> p c", p=128))
            nc.sync.dma_start(bias_v[:], bv_d.rearrange("(h d) -> d h", d=DH))
            for j in range(4):
                nc.sync.dma_start(wos[j][:], wo_d[j * 128:(j + 1) * 128, :])
            # ones columns in vh blocks (col 64 of each 66-block); pad col 0
            for t in range(16):
                blocks = vh[t][:].rearrange("p (h c) -> p h c", c=VB)
                nc.vector.memset(blocks[:, :, DH:DH + 1], 1.0)
                nc.vector.memset(blocks[:, :, DH + 1:], 0.0)

            # ---------------- projections ----------------
            with (
                tc.tile_pool(name="xq_p", bufs=12) as xq_pool,
                tc.tile_pool(name="xkv_p", bufs=12) as xkv_pool,
                tc.tile_pool(name="wqkv", bufs=1) as w_pool,
                tc.tile_pool(name="proj_ps", bufs=4, space="PSUM") as proj_ps,
            ):
                wqs = [w_pool.tile([128, JC], BF16, name=f"wqs{e}")
                       for e in range(8)]
                wks = [w_pool.tile([128, JC], BF16, name=f"wks{e}")
                       for e in range(8)]
                wvs = [w_pool.tile([128, JC], BF16, name=f"wvs{e}")
                       for e in range(8)]
                for e in range(8):
                    nc.sync.dma_start(wqs[e][:], wq_d[e * 128:(e + 1) * 128, :])
                    nc.sync.dma_start(wks[e][:], wk_d[e * 128:(e + 1) * 128, :])
                    nc.sync.dma_start(wvs[e][:], wv_d[e * 128:(e + 1) * 128, :])

                for sc in range(4):
                    ssl = slice(sc * 512, (sc + 1) * 512)
                    xqs = []
                    xks = []
                    for e in range(8):
                        xt = xq_pool.tile([128, 512], BF16,
                                          tag="xq", name="xq")
                        nc.sync.dma_start(xt[:], xq[e * 128:(e + 1) * 128, ssl])
                        xqs.append(xt)
                        kt = xkv_pool.tile([128, 512], BF16,
                                           tag="xkv", name="xkv")
                        nc.sync.dma_start(kt[:],
                                          xkv[e * 128:(e + 1) * 128, ssl])
                        xks.append(kt)
                    # Q and K projections (j-major)
                    for dst, ws, xs, bias in (
                        (qht, wqs, xqs, bias_q),
                        (kht, wks, xks, bias_k),
                    ):
                        for jc in range(4):
                            ps = proj_ps.tile([128, 512], F32,
                                              tag="pps", name="pps")
                            jsl = slice(jc * 128, (jc + 1) * 128)
                            for e in range(8):
                                nc.tensor.matmul(
                                    ps[:],
                                    ws[e][:, jsl],
                                    xs[e][:],
                                    start=(e == 0),
                                    stop=(e == 7),
                                )
                            nc.vector.tensor_scalar_add(
                                dst[jc][:, ssl], ps[:], bias[:, jc:jc + 1]
                            )
                    # V projection (t-major)
                    for tl in range(4):
                        tch = sc * 4 + tl
                        ps = proj_ps.tile([128, 512], F32,
                                          tag="pps", name="pps")
                        tsl = slice(tl * 128, (tl + 1) * 128)
                        for e in range(8):
                            nc.tensor.matmul(
                                ps[:],
                                xks[e][:, tsl],
                                wvs[e][:],
                                start=(e == 0),
                                stop=(e == 7),
                            )
                        nc.vector.tensor_copy(
                            vh[tch][:].rearrange(
                                "p (h c) -> p h c", c=VB)[:, :, 0:DH],
                            ps[:].rearrange("p (h d) -> p h d", d=DH),
                        )

            # ---------------- attention ----------------
            with (
                tc.tile_pool(name="sps", bufs=2, space="PSUM") as sp_pool,
                tc.tile_pool(name="avps", bufs=1, space="PSUM") as av_pool,
                tc.tile_pool(name="pt", bufs=4) as pt_pool,
            ):
                for h in range(HPC):
                    pair = h // 2
                    off = (h % 2) * 64
                    avs = [
                        av_pool.tile([VB, 512], F32, tag=f"av{sc}",
                                     name=f"av{sc}")
                        for sc in range(4)
                    ]
                    for t in range(16):
                        lhs_s = kht[pair][off:off + 64, t * 128:(t + 1) * 128]
                        for half in range(2):
                            sp = sp_pool.tile([128, 1024], F32,
                                              tag="sp", name="sp")
                            for i in range(2):
                                sc = half * 2 + i
                                nc.tensor.matmul(
                                    sp[:, i * 512:(i + 1) * 512],
                                    lhs_s,
                                    qht[pair][off:off + 64,
                                              sc * 512:(sc + 1) * 512],
                                    start=True,
                                    stop=True,
                                )
                            ptt = pt_pool.tile([128, 1024], BF16,
                                               tag="ptt", name="ptt")
                            nc.scalar.activation(
                                ptt[:], sp[:],
                                mybir.ActivationFunctionType.Exp,
                                scale=0.125,
                            )
                            for i in range(2):
                                sc = half * 2 + i
                                nc.tensor.matmul(
                                    avs[sc][:],
                                    vh[t][:, h * VB:(h + 1) * VB],
                                    ptt[:, i * 512:(i + 1) * 512],
                                    start=(t == 0),
                                    stop=(t == 15),
                                    skip_group_check=True,
                                )
                    # normalize head: recip of D row, broadcast, mul, +bv
                    for sc in range(4):
                        drc = drec_pool.tile([66, 512], F32,
                                             tag="drc", name="drc")
                        nc.vector.reciprocal(
                            drc[64:65, :], avs[sc][DH:DH + 1, :]
                        )
                        nc.sync.dma_start(
                            dr_d[sc * 512:(sc + 1) * 512], drc[64:65, :]
                        )
                    dreb = dre_pool.tile([64, S], F32, tag="dreb", name="dreb")
                    nc.sync.dma_start(
                        dreb[:], dr_d.unsqueeze(0).broadcast_to([64, S])
                    )
                    zn_s = zns_pool.tile([64, S], BF16, tag="zn_s", name="zn_s")
                    for sc in range(4):
                        nc.vector.tensor_mul(
                            zn_s[:, sc * 512:(sc + 1) * 512],
                            avs[sc][0:DH, :],
                            dreb[:, sc * 512:(sc + 1) * 512],
                        )
                    nc.vector.tensor_scalar_add(
                        zn_s[:], zn_s[:], bias_v[:, h:h + 1]
                    )
                    nc.sync.dma_start(znorm[pair][off:off + 64, :], zn_s[:])

            # ---------------- output projection ----------------
            with (
                tc.tile_pool(name="ops", bufs=4, space="PSUM") as op_pool,
                tc.tile_pool(name="ostg", bufs=4) as ostg_pool,
            ):
                for scc in range(16):
                    psl = slice(scc * 128, (scc + 1) * 128)
                    for oc in range(2):
                        ps = op_pool.tile([128, 512], F32, tag="ops",
                                          name="ops")
                        osl = slice(oc * 512, (oc + 1) * 512)
                        for jt in range(4):
                            nc.tensor.matmul(
                                ps[:],
                                znorm[jt][:, psl],
                                wos[jt][:, osl],
                                start=(jt == 0),
                                stop=(jt == 3),
                            )
                        ostg = ostg_pool.tile([128, 512], F32, tag="ostg",
                                              name="ostg")
                        nc.vector.tensor_copy(ostg[:], ps[:])
                        nc.sync.dma_start(out_d[psl, osl], ostg[:])

    nc.compile()
    return nc


def _bf16(a):
    return np.asarray(a, np.float32).astype(ml_dtypes.bfloat16)


def _prep_inputs(q, k_and_v, wq, bq, wk, bk, wv, bv, wo):
    """Build per-core input maps."""
    in_maps = []
    for c in range(N_CORES):
        b, g = c // 2, c % 2
        hs = slice(g * HPC, (g + 1) * HPC)
        # [H, emb, d] -> [emb, H*d] for this head group
        wq_g = np.transpose(wq[hs], (1, 0, 2)).reshape(EMB, JC)
        wk_g = np.transpose(wk[hs], (1, 0, 2)).reshape(EMB, JC)
        wv_g = np.transpose(wv[hs], (1, 0, 2)).reshape(EMB, JC)
        in_maps.append({
            "xq": np.ascontiguousarray(_bf16(q[b]).T),
            "xkv": np.ascontiguousarray(_bf16(k_and_v[b]).T),
            "wq": np.ascontiguousarray(_bf16(wq_g)),
            "wk": np.ascontiguousarray(_bf16(wk_g)),
            "wv": np.ascontiguousarray(_bf16(wv_g)),
            "bq": np.ascontiguousarray(np.asarray(bq, np.float32)[hs]
                                       .reshape(JC)),
            "bk": np.ascontiguousarray(np.asarray(bk, np.float32)[hs]
                                       .reshape(JC)),
            "bv": np.ascontiguousarray(np.asarray(bv, np.float32)[hs]
                                       .reshape(JC)),
            "wo": np.ascontiguousarray(
                _bf16(wo)[g * JC:(g + 1) * JC, :]),
        })
    return in_maps


_NC_CACHE = {}


def kernel(q, k_and_v, wq, bq, wk, bk, wv, bv, wo, bo):
    if "nc" not in _NC_CACHE:
        _NC_CACHE["nc"] = build_kernel()
    nc = _NC_CACHE["nc"]
    in_maps = _prep_inputs(q, k_and_v, wq, bq, wk, bk, wv, bv, wo)
    res = run_bass_kernel_spmd(nc, in_maps, core_ids=list(range(N_CORES)))
    bo = np.asarray(bo, np.float32)
    out = np.empty((B, S, EMB), np.float32)
    for b in range(B):
        out[b] = res.results[2 * b]["out"] + res.results[2 * b + 1]["out"] + bo
    return out


# revision 11
# speedup vs baseline: 135.5126x; 1.0229x over previous
"""Multi-head self-attention TRN2 Bass kernel.

Problem: B=4, S=2048, EMB=1024, H=16 heads, dqk=dv=64.
Sharding: 8 cores; core c handles batch b=c//2 and head group g=c%2
(8 heads each). Each core computes its partial output projection
(rows of wo for its heads); host sums the two partials per batch and
adds bo.

Per-core inputs (bf16 except biases): xq_T/xkv_T [1024, 2048] (x.T),
wq/wk/wv [1024, 512], bq/bk/bv [512] f32, wo_g [512, 1024].
Output: partial out [2048, 1024] f32.

Dataflow:
  - Q/K projections j-major: QHT/KHT [512 j, 2048 s] bf16.
  - V projection t-major: VH [2048 t, 512 j] -> tiles [128, 8*66] with
    per-head blocks [64 V | 1 ones | 1 pad]; the ones column makes the
    AV matmul emit softmax denominators as row 64 of its output.
  - scores (t-major, K=64 row-tiled): S^T [128 t, 1024 s] PSUM, exp on
    ACT (scale=1/8 folded; max-free softmax, |scores| <~ 3) -> P^T bf16.
  - AV: accumulate Z~T [66, 512] over 16 t-tiles in PSUM.
  - normalize: reciprocal of D row, DMA round-trip broadcast across
    partitions, multiply + bv bias -> ZnormT [512 j, 2048 s] bf16.
  - out projection bf16 -> partial OUT [2048, 1024] f32 -> DRAM.
"""

import ml_dtypes
import numpy as np

import concourse.bass as bass
import concourse.tile as tile
from concourse import bacc, mybir
from concourse.bass_utils import run_bass_kernel_spmd

B, S, EMB, H, DH = 4, 2048, 1024, 16, 64
N_CORES = 8
HPC = H // 2          # heads per core
JC = HPC * DH         # 512: per-core projected width
VB = DH + 2           # 66: per-head V block (64 V cols + ones + pad)

F32 = mybir.dt.float32
BF16 = mybir.dt.bfloat16


def build_kernel(reps=1):
    nc = bacc.Bacc(
        "TRN2", target_bir_lowering=False, debug=False, num_devices=N_CORES
    )

    xq = nc.dram_tensor("xq", [EMB, S], BF16, kind="ExternalInput").ap()
    xkv = nc.dram_tensor("xkv", [EMB, S], BF16, kind="ExternalInput").ap()
    wq_d = nc.dram_tensor("wq", [EMB, JC], BF16, kind="ExternalInput").ap()
    wk_d = nc.dram_tensor("wk", [EMB, JC], BF16, kind="ExternalInput").ap()
    wv_d = nc.dram_tensor("wv", [EMB, JC], BF16, kind="ExternalInput").ap()
    bq_d = nc.dram_tensor("bq", [JC], F32, kind="ExternalInput").ap()
    bk_d = nc.dram_tensor("bk", [JC], F32, kind="ExternalInput").ap()
    bv_d = nc.dram_tensor("bv", [JC], F32, kind="ExternalInput").ap()
    wo_d = nc.dram_tensor("wo", [JC, EMB], BF16, kind="ExternalInput").ap()
    out_d = nc.dram_tensor("out", [S, EMB], F32, kind="ExternalOutput").ap()
    dr_d = nc.dram_tensor("dr_scratch", [S], F32).ap()  # Drecip bounce

    import contextlib

    with tile.TileContext(nc) as tc:
        with (
            tc.For_i(0, reps, 1) if reps > 1 else contextlib.nullcontext(),
            tc.tile_pool(name="persist", bufs=1) as pp,
        ):
            # persistent SBUF tensors
            qht = [pp.tile([128, S], BF16, name=f"qht{i}") for i in range(4)]
            kht = [pp.tile([128, S], BF16, name=f"kht{i}") for i in range(4)]
            vh = [pp.tile([128, HPC * VB], BF16, name=f"vh{t}")
                  for t in range(16)]
            znorm = [pp.tile([128, S], BF16, name=f"zn{i}") for i in range(4)]
            wos = [pp.tile([128, EMB], BF16, name=f"wo{j}") for j in range(4)]
            bias_q = pp.tile([128, 4], F32, name="bias_q")
            bias_k = pp.tile([128, 4], F32, name="bias_k")
            bias_v = pp.tile([64, HPC], F32, name="bias_v")  # [d, head]

            nc.sync.dma_start(bias_q[:], bq_d.rearrange("(c p) -> p c", p=128))
            nc.sync.dma_start(bias_k[:], bk_d.rearrange("(c p) -> p c", p=128))
            nc.sync.dma_start(bias_v[:], bv_d.rearrange("(h d) -> d h", d=DH))
            for j in range(4):
                nc.sync.dma_start(wos[j][:], wo_d[j * 128:(j + 1) * 128, :])
            # ones columns in vh blocks (col 64 of each 66-block); pad col 0
            for t in range(16):
                blocks = vh[t][:].rearrange("p (h c) -> p h c", c=VB)
                nc.vector.memset(blocks[:, :, DH:DH + 1], 1.0)
                nc.vector.memset(blocks[:, :, DH + 1:], 0.0)

            # ---------------- projections ----------------
            with (
                tc.tile_pool(name="xq_p", bufs=12) as xq_pool,
                tc.tile_pool(name="xkv_p", bufs=12) as xkv_pool,
                tc.tile_pool(name="wqkv", bufs=1) as w_pool,
                tc.tile_pool(name="proj_ps", bufs=4, space="PSUM") as proj_ps,
            ):
                wqs = [w_pool.tile([128, JC], BF16, name=f"wqs{e}")
                       for e in range(8)]
                wks = [w_pool.tile([128, JC], BF16, name=f"wks{e}")
                       for e in range(8)]
                wvs = [w_pool.tile([128, JC], BF16, name=f"wvs{e}")
                       for e in range(8)]
                for e in range(8):
                    nc.sync.dma_start(wqs[e][:], wq_d[e * 128:(e + 1) * 128, :])
                    nc.sync.dma_start(wks[e][:], wk_d[e * 128:(e + 1) * 128, :])
                    nc.sync.dma_start(wvs[e][:], wv_d[e * 128:(e + 1) * 128, :])

                for sc in range(4):
                    ssl = slice(sc * 512, (sc + 1) * 512)
                    xqs = []
                    xks = []
                    for e in range(8):
                        xt = xq_pool.tile([128, 512], BF16,
                                          tag="xq", name="xq")
                        nc.sync.dma_start(xt[:], xq[e * 128:(e + 1) * 128, ssl])
                        xqs.append(xt)
                        kt = xkv_pool.tile([128, 512], BF16,
                                           tag="xkv", name="xkv")
                        nc.sync.dma_start(kt[:],
                                          xkv[e * 128:(e + 1) * 128, ssl])
                        xks.append(kt)
                    # Q and K projections (j-major)
                    for dst, ws, xs, bias in (
                        (qht, wqs, xqs, bias_q),
                        (kht, wks, xks, bias_k),
                    ):
                        for jc in range(4):
                            ps = proj_ps.tile([128, 512], F32,
                                              tag="pps", name="pps")
                            jsl = slice(jc * 128, (jc + 1) * 128)
                            for e in range(8):
                                nc.tensor.matmul(
                                    ps[:],
                                    ws[e][:, jsl],
                                    xs[e][:],
                                    start=(e == 0),
                                    stop=(e == 7),
                                )
                            nc.vector.tensor_scalar_add(
                                dst[jc][:, ssl], ps[:], bias[:, jc:jc + 1]
                            )
                    # V projection (t-major)
                    for tl in range(4):
                        tch = sc * 4 + tl
                        ps = proj_ps.tile([128, 512], F32,
                                          tag="pps", name="pps")
                        tsl = slice(tl * 128, (tl + 1) * 128)
                        for e in range(8):
                            nc.tensor.matmul(
                                ps[:],
                                xks[e][:, tsl],
                                wvs[e][:],
                                start=(e == 0),
                                stop=(e == 7),
                            )
                        nc.vector.tensor_copy(
                            vh[tch][:].rearrange(
                                "p (h c) -> p h c", c=VB)[:, :, 0:DH],
                            ps[:].rearrange("p (h d) -> p h d", d=DH),
                        )

            # ---------------- attention ----------------
            with (
                tc.tile_pool(name="sps", bufs=2, space="PSUM") as sp_pool,
                tc.tile_pool(name="avps", bufs=1, space="PSUM") as av_pool,
                tc.tile_pool(name="pt", bufs=4) as pt_pool,
                tc.tile_pool(name="dreb_p", bufs=2) as dre_pool,
                tc.tile_pool(name="znsc", bufs=2) as zns_pool,
                tc.tile_pool(name="drec_p", bufs=8) as drec_pool,
            ):
                for h in range(HPC):
                    pair = h // 2
                    off = (h % 2) * 64
                    avs = [
                        av_pool.tile([VB, 512], F32, tag=f"av{sc}",
                                     name=f"av{sc}")
                        for sc in range(4)
                    ]
                    for t in range(16):
                        lhs_s = kht[pair][off:off + 64, t * 128:(t + 1) * 128]
                        for half in range(2):
                            sp = sp_pool.tile([128, 1024], F32,
                                              tag="sp", name="sp")
                            for i in range(2):
                                sc = half * 2 + i
                                nc.tensor.matmul(
                                    sp[:, i * 512:(i + 1) * 512],
                                    lhs_s,
                                    qht[pair][off:off + 64,
                                              sc * 512:(sc + 1) * 512],
                                    start=True,
                                    stop=True,
                                )
                            ptt = pt_pool.tile([128, 1024], BF16,
                                               tag="ptt", name="ptt")
                            nc.scalar.activation(
                                ptt[:], sp[:],
                                mybir.ActivationFunctionType.Exp,
                                scale=0.125,
                            )
                            for i in range(2):
                                sc = half * 2 + i
                                nc.tensor.matmul(
                                    avs[sc][:],
                                    vh[t][:, h * VB:(h + 1) * VB],
                                    ptt[:, i * 512:(i + 1) * 512],
                                    start=(t == 0),
                                    stop=(t == 15),
                                    skip_group_check=True,
                                )
                    # normalize head: recip of D row, broadcast, mul, +bv
                    dreb = dre_pool.tile([64, S], F32, tag="dreb", name="dreb")
                    for sc in range(4):
                        drc = drec_pool.tile([66, 512], F32,
                                             tag="drc", name="drc")
                        nc.vector.reciprocal(
                            drc[64:65, :], avs[sc][DH:DH + 1, :]
                        )
                        nc.sync.dma_start(
                            dr_d[sc * 512:(sc + 1) * 512], drc[64:65, :]
                        )
                    nc.sync.dma_start(
                        dreb[:], dr_d.unsqueeze(0).broadcast_to([64, S])
                    )
                    zn_s = zns_pool.tile([64, S], BF16, tag="zn_s", name="zn_s")
                    for sc in range(4):
                        nc.vector.tensor_mul(
                            zn_s[:, sc * 512:(sc + 1) * 512],
                            avs[sc][0:DH, :],
                            dreb[:, sc * 512:(sc + 1) * 512],
                        )
                    nc.vector.tensor_scalar_add(
                        zn_s[:], zn_s[:], bias_v[:, h:h + 1]
                    )
                    nc.sync.dma_start(znorm[pair][off:off + 64, :], zn_s[:])

            # ---------------- output projection ----------------
            with (
                tc.tile_pool(name="ops", bufs=4, space="PSUM") as op_pool,
                tc.tile_pool(name="ostg", bufs=4) as ostg_pool,
            ):
                for scc in range(16):
                    psl = slice(scc * 128, (scc + 1) * 128)
                    for oc in range(2):
                        ps = op_pool.tile([128, 512], F32, tag="ops",
                                          name="ops")
                        osl = slice(oc * 512, (oc + 1) * 512)
                        for jt in range(4):
                            nc.tensor.matmul(
                                ps[:],
                                znorm[jt][:, psl],
                                wos[jt][:, osl],
                                start=(jt == 0),
                                stop=(jt == 3),
                            )
                        ostg = ostg_pool.tile([128, 512], F32, tag="ostg",
                                              name="ostg")
                        nc.vector.tensor_copy(ostg[:], ps[:])
                        nc.sync.dma_start(out_d[psl, osl], ostg[:])

    nc.compile()
    return nc


def _bf16(a):
    return np.asarray(a, np.float32).astype(ml_dtypes.bfloat16)


def _prep_inputs(q, k_and_v, wq, bq, wk, bk, wv, bv, wo):
    """Build per-core input maps."""
    in_maps = []
    for c in range(N_CORES):
        b, g = c // 2, c % 2
        hs = slice(g * HPC, (g + 1) * HPC)
        # [H, emb, d] -> [emb, H*d] for this head group
        wq_g = np.transpose(wq[hs], (1, 0, 2)).reshape(EMB, JC)
        wk_g = np.transpose(wk[hs], (1, 0, 2)).reshape(EMB, JC)
        wv_g = np.transpose(wv[hs], (1, 0, 2)).reshape(EMB, JC)
        in_maps.append({
            "xq": np.ascontiguousarray(_bf16(q[b]).T),
            "xkv": np.ascontiguousarray(_bf16(k_and_v[b]).T),
            "wq": np.ascontiguousarray(_bf16(wq_g)),
            "wk": np.ascontiguousarray(_bf16(wk_g)),
            "wv": np.ascontiguousarray(_bf16(wv_g)),
            "bq": np.ascontiguousarray(np.asarray(bq, np.float32)[hs]
                                       .reshape(JC)),
            "bk": np.ascontiguousarray(np.asarray(bk, np.float32)[hs]
                                       .reshape(JC)),
            "bv": np.ascontiguousarray(np.asarray(bv, np.float32)[hs]
                                       .reshape(JC)),
            "wo": np.ascontiguousarray(
                _bf16(wo)[g * JC:(g + 1) * JC, :]),
        })
    return in_maps


_NC_CACHE = {}


def kernel(q, k_and_v, wq, bq, wk, bk, wv, bv, wo, bo):
    if "nc" not in _NC_CACHE:
        _NC_CACHE["nc"] = build_kernel()
    nc = _NC_CACHE["nc"]
    in_maps = _prep_inputs(q, k_and_v, wq, bq, wk, bk, wv, bv, wo)
    res = run_bass_kernel_spmd(nc, in_maps, core_ids=list(range(N_CORES)))
    bo = np.asarray(bo, np.float32)
    out = np.empty((B, S, EMB), np.float32)
    for b in range(B):
        out[b] = res.results[2 * b]["out"] + res.results[2 * b + 1]["out"] + bo
    return out
